# revision 32
# baseline (speedup 1.0000x reference)
"""Trainium2 Bass kernel for InteractorwoLSTM additive attention.

out[b,t,:] = alpha[b,t,:] @ h_s[b]  with
  beta[b,t,n] = W_w . tanh(h_s[b,n]@W_S + b_S + h_v[b,t]@W_V + b_V) + b_w
  alpha = masked-softmax(beta) per reference semantics.

Design: length-truncated slots + blob DMA + PSUM-accumulated beta.
Measured ~54us on HW (baseline 82.4us); ACT tanh stream runs at ~100%
occupancy, so the kernel sits on the ACT roofline for its element count.

Positions n >= lengths[b] never influence the output (the reference's
masked-softmax renormalization cancels them), so the tanh/add/beta work
for those positions is skipped. lengths are known on the host before
compile, so the program is built for the actual length profile:
batches sorted by length desc; slot k on core c holds sorted rank
8k+c; slot k's n-bound = L(rank 8k) (SPMD shares one program). For the
seed-0 data: bounds [30,17,12,8] -> 67 packed n-columns vs 120
(~44% less work on ACT -- the bottleneck engine at 1 elem/cycle/lane).

Ramp design (the first ~8us are framework preamble + DMA):
  - descriptor generation costs ~0.65us per dma_start, serialized on
    the ISSUING sequencer (and modeled so by the tile scheduler's
    CoreSim -- a dma_start on the scalar queue serializes against ACT
    dispatch and makes the scheduler statically sink dependent work).
    So: no input DMA ever rides the scalar queue; S-side pieces ride
    sync, V-side pieces ride gpsimd, first-granule deps first; the
    first pieces are host-packed blobs (A1 = hsT|WS0, A3 = WV0|Ww|bwm)
    with 1.5-3.9KB contiguous per-partition runs.
  - slot-0 V projections are per-chunk so the tanh cadence never waits
    for other slots' hvT; slots 1-3 V projections batched+deferred.
  - slot-0 beta matmuls are emitted one chunk late so the PE queue
    never blocks a projection behind tanh-gated work.
  - b_w and the pad-kill fold into one per-column bias (bwm: b_w on
    valid cols, -50 on pads -> exp underflows, no mask mult).
  - beta accumulates across all 4 D-chunks into one [128, bound] PSUM
    plane (slot's first matmul start=True clears the bank; the rest
    accumulate per-element via has_written), so the softmax reads one
    qa = beta + bwm add straight from PSUM.
  - last granule's tanh splits by n-halves; the last out-scale runs on
    ACT (idle at the tail, faster PSUM reads); dummy tanh at t=0 pulls
    the ACT table load under the input DMAs.
"""

import numpy as np

B, T, N = 32, 128, 30
D = 512
NCORES = 8
BPC = B // NCORES  # batch slots per core
NC = D // 128  # 4 chunks of 128 along D

_CACHE = {}


def _build(bounds):
    import concourse.bacc as bacc
    import concourse.tile as tile
    from concourse import mybir
    from concourse.masks import make_identity

    f32 = mybir.dt.float32
    bf16 = mybir.dt.bfloat16

    offs = [0]
    for b in bounds:
        offs.append(offs[-1] + b)
    P = offs[-1]
    B0 = bounds[0]
    # blob column layouts (bf16, [128, X])
    XA1 = NC * P + NC * 128  # hsT | WS0
    XA2 = NC * 128 + NC * 128 + NC + P  # hvT0 | WV0 | Ww | bwm

    nc = bacc.Bacc(
        "TRN2",
        target_bir_lowering=False,
        debug=False,
        enable_asserts=False,
        num_devices=NCORES,
    )

    # ---- DRAM I/O (host-packed blobs; see _make_in_maps) ----
    A1_d = nc.dram_tensor("A1", [128, XA1], bf16, kind="ExternalInput").ap()
    A2_d = nc.dram_tensor("A2", [128, XA2], bf16, kind="ExternalInput").ap()
    WSr_d = nc.dram_tensor("WSr", [NC - 1, 128, NC, 128], bf16, kind="ExternalInput").ap()
    WVr_d = nc.dram_tensor("WVr", [NC - 1, 128, NC, 128], bf16, kind="ExternalInput").ap()
    hvTr_d = nc.dram_tensor("hvTr", [128, BPC - 1, NC, 128], bf16, kind="ExternalInput").ap()
    bSV_d = nc.dram_tensor("bSV", [1, D], bf16, kind="ExternalInput").ap()
    hs_d = nc.dram_tensor("hs", [B0, BPC, D], bf16, kind="ExternalInput").ap()
    out_d = nc.dram_tensor("out", [BPC, T, D], bf16, kind="ExternalOutput").ap()

    with tile.TileContext(nc) as tc:
        with (
            tc.tile_pool(name="const", bufs=1) as const,
            tc.tile_pool(name="epre", bufs=3) as eprep,
            tc.tile_pool(name="epre2", bufs=3) as eprep2,
            tc.tile_pool(name="ebig", bufs=1) as ebigp,
            tc.tile_pool(name="soft", bufs=3) as softp,
            tc.tile_pool(name="outp", bufs=3) as outp,
            tc.tile_pool(name="pv", bufs=2, space="PSUM") as pvp,
            tc.tile_pool(name="ps", bufs=2, space="PSUM") as psp,
            tc.tile_pool(name="pbeta", bufs=2, space="PSUM") as pbetap,
            tc.tile_pool(name="pqt", bufs=1, space="PSUM") as pqtp,
            tc.tile_pool(name="pfin", bufs=1, space="PSUM") as pfinp,
        ):
            # ---- force the exp/tanh ACT table load to start at t=0 so it
            # hides under the input DMAs (the set covers tanh+exp+copy)
            warm = const.tile([1, 2], f32)
            nc.vector.memset(warm[:], 0.0)
            warm2 = const.tile([1, 2], f32)
            nc.scalar.activation(warm2[:], warm[:], mybir.ActivationFunctionType.Tanh)

            A1_sb = const.tile([128, XA1], bf16)
            A2_sb = const.tile([128, XA2], bf16)
            WSr_sb = const.tile([128, NC - 1, NC, 128], bf16)
            WVr_sb = const.tile([128, NC - 1, NC, 128], bf16)
            hvTr_sb = const.tile([128, BPC - 1, NC, 128], bf16)
            bSV_sb = const.tile([1, D], bf16)
            hs_sb = const.tile([B0, BPC, D], bf16)

            # dma_start occupies its ISSUING engine (~0.5-0.7us descriptor
            # gen, both in the scheduler's CoreSim and on the real
            # sequencer). The scalar queue shares the ACT dispatch path, so
            # input DMAs must NEVER ride it -- S-side pieces go on sync
            # (SP is otherwise idle), V-side pieces on gpsimd, each queue
            # in consumption order with the first-granule deps leading.
            nc.sync.dma_start(out=A1_sb[:], in_=A1_d)
            nc.gpsimd.dma_start(out=A2_sb[:], in_=A2_d)
            nc.sync.dma_start(out=bSV_sb[:], in_=bSV_d)
            nc.sync.dma_start(out=WSr_sb[:, 0, :, :], in_=WSr_d[0])
            nc.gpsimd.dma_start(out=WVr_sb[:, 0, :, :], in_=WVr_d[0])
            nc.sync.dma_start(out=WSr_sb[:, 1, :, :], in_=WSr_d[1])
            nc.gpsimd.dma_start(out=WVr_sb[:, 1, :, :], in_=WVr_d[1])
            nc.sync.dma_start(out=WSr_sb[:, 2, :, :], in_=WSr_d[2])
            nc.gpsimd.dma_start(out=WVr_sb[:, 2, :, :], in_=WVr_d[2])
            nc.sync.dma_start(out=hvTr_sb[:], in_=hvTr_d)
            nc.sync.dma_start(out=hs_sb[:], in_=hs_d)

            # views into the blobs
            hsT_v = A1_sb[:, 0 : NC * P].rearrange("p (k x) -> p k x", k=NC)
            WS0_v = A1_sb[:, NC * P :].rearrange("p (k x) -> p k x", k=NC)
            hvT0_v = A2_sb[:, 0 : NC * 128].rearrange("p (k x) -> p k x", k=NC)
            WV0_v = A2_sb[:, NC * 128 : 2 * NC * 128].rearrange(
                "p (k x) -> p k x", k=NC
            )
            Ww_v = A2_sb[:, 2 * NC * 128 : 2 * NC * 128 + NC]
            bwm_v = A2_sb[:, 2 * NC * 128 + NC :]

            def WSv(mc, kc):
                return WS0_v[:, kc, :] if mc == 0 else WSr_sb[:, mc - 1, kc, :]

            def WVv(mc, kc):
                return WV0_v[:, kc, :] if mc == 0 else WVr_sb[:, mc - 1, kc, :]

            ident = const.tile([128, 128], f32)
            make_identity(nc, ident[:])
            onesP = const.tile([1, P], bf16)
            nc.vector.memset(onesP[:], 1.0)

            VT_sb = const.tile([128, BPC, NC, 128], bf16)
            ST_dup = const.tile([128, NC, P, 2], bf16)

            # ---- helpers -------------------------------------------------
            def proj_S(mc):
                # S chunk: ST'[d, packed(slot, n)] for all slots
                ps_s = psp.tile([128, P], f32, tag="ps")
                for kc in range(NC):
                    nc.tensor.matmul(
                        ps_s[:],
                        WSv(mc, kc),
                        hsT_v[:, kc, :],
                        start=(kc == 0),
                        stop=False,
                    )
                # + (b_S + b_V) broadcast along packed cols: rank-1 K=1 matmul
                nc.tensor.matmul(
                    ps_s[:],
                    bSV_sb[0:1, mc * 128 : (mc + 1) * 128],
                    onesP[0:1, :],
                    start=False,
                    stop=True,
                )
                # ST_dup[d, mc, p, 2] <- ps_s duplicated over pair axis; for
                # chunk 0 split slot 0 out so the head of the ACT chain isn't
                # gated on the full-P cast
                if mc == 0:
                    nc.vector.tensor_copy(
                        ST_dup[:, mc, 0 : offs[1], :],
                        ps_s[:, 0 : offs[1]]
                        .unsqueeze(2)
                        .broadcast_to([128, bounds[0], 2]),
                    )
                    nc.vector.tensor_copy(
                        ST_dup[:, mc, offs[1] :, :],
                        ps_s[:, offs[1] :]
                        .unsqueeze(2)
                        .broadcast_to([128, P - offs[1], 2]),
                    )
                else:
                    nc.vector.tensor_copy(
                        ST_dup[:, mc, :, :],
                        ps_s[:].unsqueeze(2).broadcast_to([128, P, 2]),
                    )

            def proj_V0(mc):
                # V chunk for slot 0 only (gated just on hvT0 + WV_mc)
                pv_t = pvp.tile([128, BPC, 128], f32, tag="pv")
                for kc in range(NC):
                    nc.tensor.matmul(
                        pv_t[:, 0, :],
                        WVv(mc, kc),
                        hvT0_v[:, kc, :],
                        start=(kc == 0),
                        stop=(kc == NC - 1),
                    )
                nc.vector.tensor_copy(VT_sb[:, 0, mc, :], pv_t[:, 0, :])

            def proj_Vr(mc):
                # V chunk for slots 1..3, batched in the rhs free dim
                pv_t = pvp.tile([128, BPC, 128], f32, tag="pv")
                for kc in range(NC):
                    nc.tensor.matmul(
                        pv_t[:, 1:BPC, :],
                        WVv(mc, kc),
                        hvTr_sb[:, :, kc, :],
                        start=(kc == 0),
                        stop=(kc == NC - 1),
                    )
                nc.vector.tensor_copy(VT_sb[:, 1:BPC, mc, :], pv_t[:, 1:BPC, :])

            def ep_add(k, c, ep_slice, n0, n1):
                """e_pre = VT (+bcast over n) + ST' (+bcast over t-pairs)
                for slot k chunk c, rows [n0,n1), into ep_slice."""
                nn = n1 - n0
                nc.vector.tensor_add(
                    ep_slice.rearrange("p n (t two) -> p n t two", two=2),
                    VT_sb[:, k, c, :]
                    .rearrange("p (t two) -> p t two", two=2)
                    .unsqueeze(1)
                    .broadcast_to([128, nn, 64, 2]),
                    ST_dup[:, c, offs[k] + n0 : offs[k] + n1, :]
                    .unsqueeze(2)
                    .broadcast_to([128, nn, 64, 2]),
                )

            def beta_mms(k, c, eb, beta_acc, n0, n1):
                # all 4 chunks accumulate into one [128, bound] PSUM plane:
                # the slot's first matmul (c0,n0) start=True clears the bank's
                # has_written bits; every later matmul accumulates (c>0, bits
                # set) or overwrites (c==0, bits clear) per element
                bk = bounds[k]
                for n in range(n0, n1):
                    nc.tensor.matmul(
                        beta_acc[:, n : n + 1],
                        eb[:, c, n, :],
                        Ww_v[:, c : c + 1],
                        start=(c == 0 and n == 0),
                        stop=(c == NC - 1 and n == bk - 1),
                    )

            def add_tanh(k, c, eb):
                """add (DVE 2x) -> tanh (ACT) for one chunk; betas deferred."""
                bk = bounds[k]
                ep = eprep.tile([128, B0, 128], bf16, tag="ep")
                ep_add(k, c, ep[:, 0:bk, :], 0, bk)
                nc.scalar.activation(
                    eb[:, c, :, :], ep[:, 0:bk, :], mybir.ActivationFunctionType.Tanh
                )

            def add_tanh_head(k, c, eb, beta_big):
                """First granule: add/tanh split on an accelerating n-grid so
                the ACT chain starts on a tiny first dependency while each
                later add still finishes before the previous tanh ends."""
                bk = bounds[k]
                cuts = sorted({0, min(4, bk), min(12, bk), min(21, bk), bk})
                ep = eprep.tile([128, B0, 128], bf16, tag="ep")
                for n0, n1 in zip(cuts[:-1], cuts[1:]):
                    if n1 <= n0:
                        continue
                    ep_add(k, c, ep[:, n0:n1, :], n0, n1)
                    nc.scalar.activation(
                        eb[:, c, n0:n1, :],
                        ep[:, n0:n1, :],
                        mybir.ActivationFunctionType.Tanh,
                    )

            def granule2(k, c0, eb, beta_big, tail=False):
                """2-chunk granule: two adds, one tanh, betas for both chunks.
                tail=True splits the second chunk's tanh by n-halves so the
                final beta matmuls and softmax start earlier."""
                bk = bounds[k]
                ep = eprep2.tile([128, 2, bounds[1], 128], bf16, tag="ep2")
                ep_add(k, c0, ep[:, 0, 0:bk, :], 0, bk)
                ep_add(k, c0 + 1, ep[:, 1, 0:bk, :], 0, bk)
                if not tail:
                    nc.scalar.activation(
                        eb[:, c0 : c0 + 2, :, :],
                        ep[:, :, 0:bk, :],
                        mybir.ActivationFunctionType.Tanh,
                    )
                    beta_mms(k, c0, eb, beta_big, 0, bk)
                    beta_mms(k, c0 + 1, eb, beta_big, 0, bk)
                else:
                    h = max(1, bk // 2)
                    nc.scalar.activation(
                        eb[:, c0, :, :],
                        ep[:, 0, 0:bk, :],
                        mybir.ActivationFunctionType.Tanh,
                    )
                    beta_mms(k, c0, eb, beta_big, 0, bk)
                    nc.scalar.activation(
                        eb[:, c0 + 1, 0:h, :],
                        ep[:, 1, 0:h, :],
                        mybir.ActivationFunctionType.Tanh,
                    )
                    beta_mms(k, c0 + 1, eb, beta_big, 0, h)
                    if h < bk:
                        nc.scalar.activation(
                            eb[:, c0 + 1, h:bk, :],
                            ep[:, 1, h:bk, :],
                            mybir.ActivationFunctionType.Tanh,
                        )
                        beta_mms(k, c0 + 1, eb, beta_big, h, bk)

            def softmax_final(k):
                bk = bounds[k]
                beta_acc = slot_tiles[k][1]
                qa = softp.tile([128, B0], f32, tag="qa")
                # + bwm: b_w on valid cols, -50 on pad cols (exp kills pads)
                nc.vector.tensor_add(
                    qa[:, 0:bk],
                    beta_acc[:, 0:bk],
                    bwm_v[:, offs[k] : offs[k] + bk],
                )
                t1 = softp.tile([128, B0], f32, tag="t1")
                nc.scalar.activation(
                    t1[:, 0:bk], qa[:, 0:bk], mybir.ActivationFunctionType.Exp
                )
                Qs = softp.tile([128, 1], f32, tag="Z1")
                nc.vector.tensor_reduce(
                    Qs[:], t1[:, 0:bk], mybir.AxisListType.X, mybir.AluOpType.add
                )
                recip = softp.tile([128, 1], f32, tag="recip")
                nc.vector.reciprocal(recip[:], Qs[:])
                # ---- out[k] = (t1 @ h_s_masked[k]) * recip ----
                qT_ps = pqtp.tile([B0, 128], f32, tag="qt")
                nc.tensor.transpose(qT_ps[0:bk, :], t1[:, 0:bk], ident[:])
                qT = softp.tile([B0, 128], bf16, tag="qTs")
                nc.vector.tensor_copy(qT[0:bk, :], qT_ps[0:bk, :])
                out_ps = pfinp.tile([128, D], f32, tag="out")
                out_sb = outp.tile([128, D], bf16, tag="osb")
                nc.tensor.matmul(
                    out_ps[:], qT[0:bk, :], hs_sb[0:bk, k, :], start=True, stop=True
                )
                if k == BPC - 1:
                    # tail: ACT is idle by now and reads PSUM faster than DVE
                    nc.scalar.activation(
                        out_sb[:],
                        out_ps[:],
                        mybir.ActivationFunctionType.Copy,
                        scale=recip[:],
                    )
                else:
                    nc.vector.tensor_scalar_mul(out_sb[:], out_ps[:], recip[:])
                nc.sync.dma_start(out=out_d[k], in_=out_sb[:])

            # ---- slot 0 interleaved with projections; beta matmuls are
            # emitted one chunk late so the PE queue never blocks a
            # projection behind tanh-gated work ----
            slot_tiles = {}

            def alloc_slot(k):
                slot_tiles[k] = (
                    ebigp.tile(
                        [128, NC, bounds[k], 128], bf16, tag=f"e{k}", name=f"eb{k}"
                    ),
                    pbetap.tile([128, bounds[k]], f32, tag="beta", name=f"bb{k}"),
                )

            for mc in range(NC):
                proj_S(mc)
                proj_V0(mc)
                if mc == 0:
                    alloc_slot(0)
                    add_tanh_head(0, 0, *slot_tiles[0])
                else:
                    beta_mms(0, mc - 1, *slot_tiles[0], 0, bounds[0])
                    add_tanh(0, mc, slot_tiles[0][0])
            for mc in range(NC):
                proj_Vr(mc)
            beta_mms(0, NC - 1, *slot_tiles[0], 0, bounds[0])

            # ---- remaining slots, softmax/final pipelined one slot late ----
            for k in range(1, BPC):
                alloc_slot(k)
                eb, beta_big = slot_tiles[k]
                last = k == BPC - 1
                if not last:
                    granule2(k, 0, eb, beta_big)
                    softmax_final(k - 1)
                    granule2(k, 2, eb, beta_big)
                else:
                    # last slot: softmax first so its DVE ops don't sit
                    # between this slot's adds (the end is the critical tail)
                    softmax_final(k - 1)
                    granule2(k, 0, eb, beta_big)
                    granule2(k, 2, eb, beta_big, tail=True)
            softmax_final(BPC - 1)

    nc.compile()
    return nc


def _get_nc(bounds):
    key = tuple(bounds)
    if key not in _CACHE:
        _CACHE[key] = _build(list(bounds))
    return _CACHE[key]


def _plan(lengths):
    """Sort batches by length desc; slot k on core c <- sorted rank 8k+c.
    Returns (order, bounds)."""
    lengths = np.asarray(lengths).reshape(-1)
    order = np.argsort(-lengths, kind="stable")
    bounds = [int(lengths[order[NCORES * k]]) for k in range(BPC)]
    return order, bounds


def _make_in_maps(order, bounds, h_s, h_v, lengths, W_S, b_S, W_V, b_V, W_w, b_w):
    f32 = np.float32
    h_s = np.asarray(h_s, dtype=f32)
    h_v = np.asarray(h_v, dtype=f32)
    lengths = np.asarray(lengths).reshape(-1)
    offs = np.concatenate([[0], np.cumsum(bounds)]).astype(int)
    P = int(offs[-1])
    B0 = bounds[0]

    # weights, chunked + cast once (shared across cores); mc outermost
    WS = np.ascontiguousarray(
        np.asarray(W_S, f32).reshape(NC, 128, NC, 128).transpose(2, 1, 0, 3)
    )  # [mc, p, kc, 128]
    WV = np.ascontiguousarray(
        np.asarray(W_V, f32).reshape(NC, 128, NC, 128).transpose(2, 1, 0, 3)
    )
    Ww = np.ascontiguousarray(np.asarray(W_w, f32).reshape(NC, 128).T)  # [128, NC]
    bSV = (np.asarray(b_S, f32) + np.asarray(b_V, f32)).reshape(1, D)
    bw_val = f32(np.asarray(b_w).reshape(-1)[0])

    try:
        import ml_dtypes

        bf16 = ml_dtypes.bfloat16
    except ImportError:
        import jax.numpy as jnp

        bf16 = jnp.bfloat16

    def to_bf16(x):
        return np.asarray(x, dtype=bf16)

    WS_b = to_bf16(WS)
    WV_b = to_bf16(WV)
    Ww_b = to_bf16(Ww)
    bSV_b = to_bf16(bSV)
    WSr_b = np.ascontiguousarray(WS_b[1:])
    WVr_b = np.ascontiguousarray(WV_b[1:])

    in_maps = []
    for core in range(NCORES):
        batches = [int(order[NCORES * k + core]) for k in range(BPC)]
        hv_c = h_v[batches]  # (BPC, T, D)
        hvT = np.ascontiguousarray(
            hv_c.reshape(BPC, T, NC, 128).transpose(0, 3, 2, 1)
        )  # (slot, 128p, kc, t)
        hvT_b = to_bf16(hvT)
        hsT = np.zeros((128, NC, P), dtype=f32)
        hs_r = np.zeros((B0, BPC, D), dtype=f32)  # (n, slot, D), masked rows 0
        bwm = np.full((128, P), -50.0, dtype=f32)
        for k, b in enumerate(batches):
            L = int(lengths[b])
            bk = bounds[k]
            Lk = min(L, bk)
            hk = h_s[b, :Lk]  # (Lk, D)
            hsT[:, :, offs[k] : offs[k] + Lk] = hk.reshape(Lk, NC, 128).transpose(
                2, 1, 0
            )
            hs_r[:Lk, k, :] = hk
            bwm[:, offs[k] : offs[k] + Lk] = bw_val
        # blobs: A1 = hsT | WS0 ; A2 = hvT0 | WV0 | Ww | bwm  (all bf16)
        A1 = np.concatenate(
            [to_bf16(hsT).reshape(128, NC * P), WS_b[0].reshape(128, NC * 128)],
            axis=1,
        )
        A2 = np.concatenate(
            [
                hvT_b[0].reshape(128, NC * 128),
                WV_b[0].reshape(128, NC * 128),
                Ww_b,
                to_bf16(bwm),
            ],
            axis=1,
        )
        in_maps.append(
            {
                "A1": np.ascontiguousarray(A1),
                "A2": np.ascontiguousarray(A2),
                "WSr": WSr_b,
                "WVr": WVr_b,
                "hvTr": np.ascontiguousarray(hvT_b[1:].transpose(1, 0, 2, 3)),
                "bSV": bSV_b,
                "hs": to_bf16(hs_r),
            }
        )
    return in_maps


def run(inputs: dict, trace: bool = False):
    """Run on 8 NeuronCores; returns (output, BassKernelResults)."""
    from concourse import bass_utils

    order, bounds = _plan(inputs["lengths"])
    nc = _get_nc(bounds)
    in_maps = _make_in_maps(order, bounds, **inputs)
    res = bass_utils.run_bass_kernel_spmd(
        nc, in_maps, core_ids=list(range(NCORES)), trace=trace
    )
    full = np.zeros((B, T, D), dtype=np.float32)
    for core in range(NCORES):
        o = np.asarray(res.results[core]["out"], dtype=np.float32)
        for k in range(BPC):
            full[int(order[NCORES * k + core])] = o[k]
    return full, res


def kernel(**inputs) -> np.ndarray:
    out, _ = run(inputs, trace=False)
    return out


# revision 34
# speedup vs baseline: 1.1751x; 1.1751x over previous
"""Trainium2 Bass kernel for InteractorwoLSTM additive attention.

out[b,t,:] = alpha[b,t,:] @ h_s[b]  with
  beta[b,t,n] = W_w . tanh(h_s[b,n]@W_S + b_S + h_v[b,t]@W_V + b_V) + b_w
  alpha = masked-softmax(beta) per reference semantics.

Design: length-truncated slots + blob DMA + PSUM-accumulated beta.
Measured ~54us on HW (baseline 82.4us); ACT tanh stream runs at ~100%
occupancy, so the kernel sits on the ACT roofline for its element count.

Positions n >= lengths[b] never influence the output (the reference's
masked-softmax renormalization cancels them), so the tanh/add/beta work
for those positions is skipped. lengths are known on the host before
compile, so the program is built for the actual length profile:
batches sorted by length desc; slot k on core c holds sorted rank
8k+c; slot k's n-bound = L(rank 8k) (SPMD shares one program). For the
seed-0 data: bounds [30,17,12,8] -> 67 packed n-columns vs 120
(~44% less work on ACT -- the bottleneck engine at 1 elem/cycle/lane).

Ramp design (the first ~8us are framework preamble + DMA):
  - descriptor generation costs ~0.65us per dma_start, serialized on
    the ISSUING sequencer (and modeled so by the tile scheduler's
    CoreSim -- a dma_start on the scalar queue serializes against ACT
    dispatch and makes the scheduler statically sink dependent work).
    So: no input DMA ever rides the scalar queue; S-side pieces ride
    sync, V-side pieces ride gpsimd, first-granule deps first; the
    first pieces are host-packed blobs (A1 = hsT|WS0, A3 = WV0|Ww|bwm)
    with 1.5-3.9KB contiguous per-partition runs.
  - slot-0 V projections are per-chunk so the tanh cadence never waits
    for other slots' hvT; slots 1-3 V projections batched+deferred.
  - slot-0 beta matmuls are emitted one chunk late so the PE queue
    never blocks a projection behind tanh-gated work.
  - b_w and the pad-kill fold into one per-column bias (bwm: b_w on
    valid cols, -50 on pads -> exp underflows, no mask mult).
  - beta accumulates across all 4 D-chunks into one [128, bound] PSUM
    plane (slot's first matmul start=True clears the bank; the rest
    accumulate per-element via has_written), so the softmax reads one
    qa = beta + bwm add straight from PSUM.
  - last granule's tanh splits by n-halves; the last out-scale runs on
    ACT (idle at the tail, faster PSUM reads); dummy tanh at t=0 pulls
    the ACT table load under the input DMAs.
"""

import numpy as np

B, T, N = 32, 128, 30
D = 512
NCORES = 8
BPC = B // NCORES  # batch slots per core
NC = D // 128  # 4 chunks of 128 along D

_CACHE = {}


def _build(bounds):
    import concourse.bacc as bacc
    import concourse.tile as tile
    from concourse import mybir
    from concourse.masks import make_identity

    f32 = mybir.dt.float32
    bf16 = mybir.dt.bfloat16

    offs = [0]
    for b in bounds:
        offs.append(offs[-1] + b)
    P = offs[-1]
    B0 = bounds[0]
    # blob column layouts (bf16, [128, X])
    XA1 = NC * P + NC * 128  # hsT | WS0
    XA3 = NC * 128 + NC + P  # WV0 | Ww | bwm

    nc = bacc.Bacc(
        "TRN2",
        target_bir_lowering=False,
        debug=False,
        enable_asserts=False,
        num_devices=NCORES,
    )

    # ---- DRAM I/O (host-packed blobs; see _make_in_maps) ----
    A1_d = nc.dram_tensor("A1", [128, XA1], bf16, kind="ExternalInput").ap()
    hvT0_d = nc.dram_tensor("hvT0", [128, NC, 128], bf16, kind="ExternalInput").ap()
    A3_d = nc.dram_tensor("A3", [128, XA3], bf16, kind="ExternalInput").ap()
    WSr_d = nc.dram_tensor("WSr", [NC - 1, 128, NC, 128], bf16, kind="ExternalInput").ap()
    WVr_d = nc.dram_tensor("WVr", [NC - 1, 128, NC, 128], bf16, kind="ExternalInput").ap()
    hvTr_d = nc.dram_tensor("hvTr", [128, BPC - 1, NC, 128], bf16, kind="ExternalInput").ap()
    bSV_d = nc.dram_tensor("bSV", [1, D], bf16, kind="ExternalInput").ap()
    hs_d = nc.dram_tensor("hs", [B0, BPC, D], bf16, kind="ExternalInput").ap()
    out_d = nc.dram_tensor("out", [BPC, T, D], bf16, kind="ExternalOutput").ap()

    with tile.TileContext(nc) as tc:
        with (
            tc.tile_pool(name="const", bufs=1) as const,
            tc.tile_pool(name="epre", bufs=3) as eprep,
            tc.tile_pool(name="epre2", bufs=3) as eprep2,
            tc.tile_pool(name="ebig", bufs=1) as ebigp,
            tc.tile_pool(name="soft", bufs=3) as softp,
            tc.tile_pool(name="outp", bufs=3) as outp,
            tc.tile_pool(name="pv", bufs=2, space="PSUM") as pvp,
            tc.tile_pool(name="ps", bufs=2, space="PSUM") as psp,
            tc.tile_pool(name="pbeta", bufs=2, space="PSUM") as pbetap,
            tc.tile_pool(name="pqt", bufs=1, space="PSUM") as pqtp,
            tc.tile_pool(name="pfin", bufs=1, space="PSUM") as pfinp,
        ):
            # ---- force the exp/tanh ACT table load to start at t=0 so it
            # hides under the input DMAs (the set covers tanh+exp+copy)
            warm = const.tile([1, 2], f32)
            nc.vector.memset(warm[:], 0.0)
            warm2 = const.tile([1, 2], f32)
            nc.scalar.activation(warm2[:], warm[:], mybir.ActivationFunctionType.Tanh)

            A1_sb = const.tile([128, XA1], bf16)
            hvT0_sb = const.tile([128, NC, 128], bf16)
            A3_sb = const.tile([128, XA3], bf16)
            WSr_sb = const.tile([128, NC - 1, NC, 128], bf16)
            WVr_sb = const.tile([128, NC - 1, NC, 128], bf16)
            hvTr_sb = const.tile([128, BPC - 1, NC, 128], bf16)
            bSV_sb = const.tile([1, D], bf16)
            hs_sb = const.tile([B0, BPC, D], bf16)

            # dma_start occupies its ISSUING engine (~0.5-0.7us descriptor
            # gen, both in the scheduler's CoreSim and on the real
            # sequencer). The scalar queue shares the ACT dispatch path, so
            # input DMAs must NEVER ride it -- S-side pieces go on sync
            # (SP is otherwise idle), V-side pieces on gpsimd, each queue
            # in consumption order with the first-granule deps leading.
            nc.sync.dma_start(out=A1_sb[:], in_=A1_d)
            nc.gpsimd.dma_start(out=hvT0_sb[:], in_=hvT0_d)
            nc.sync.dma_start(out=bSV_sb[:], in_=bSV_d)
            nc.gpsimd.dma_start(out=A3_sb[:], in_=A3_d)
            nc.sync.dma_start(out=WSr_sb[:, 0, :, :], in_=WSr_d[0])
            nc.gpsimd.dma_start(out=WVr_sb[:, 0, :, :], in_=WVr_d[0])
            nc.sync.dma_start(out=WSr_sb[:, 1, :, :], in_=WSr_d[1])
            nc.gpsimd.dma_start(out=WVr_sb[:, 1, :, :], in_=WVr_d[1])
            nc.sync.dma_start(out=WSr_sb[:, 2, :, :], in_=WSr_d[2])
            nc.gpsimd.dma_start(out=WVr_sb[:, 2, :, :], in_=WVr_d[2])
            nc.sync.dma_start(out=hvTr_sb[:], in_=hvTr_d)
            nc.sync.dma_start(out=hs_sb[:], in_=hs_d)

            # views into the blobs
            hsT_v = A1_sb[:, 0 : NC * P].rearrange("p (k x) -> p k x", k=NC)
            WS0_v = A1_sb[:, NC * P :].rearrange("p (k x) -> p k x", k=NC)
            WV0_v = A3_sb[:, 0 : NC * 128].rearrange("p (k x) -> p k x", k=NC)
            Ww_v = A3_sb[:, NC * 128 : NC * 128 + NC]
            bwm_v = A3_sb[:, NC * 128 + NC :]

            def WSv(mc, kc):
                return WS0_v[:, kc, :] if mc == 0 else WSr_sb[:, mc - 1, kc, :]

            def WVv(mc, kc):
                return WV0_v[:, kc, :] if mc == 0 else WVr_sb[:, mc - 1, kc, :]

            ident = const.tile([128, 128], f32)
            make_identity(nc, ident[:])
            onesP = const.tile([1, P], bf16)
            nc.vector.memset(onesP[:], 1.0)

            VT_sb = const.tile([128, BPC, NC, 128], bf16)
            ST_dup = const.tile([128, NC, P, 2], bf16)

            # ---- helpers -------------------------------------------------
            def proj_S(mc):
                # S chunk: ST'[d, packed(slot, n)] for all slots
                ps_s = psp.tile([128, P], f32, tag="ps")
                for kc in range(NC):
                    nc.tensor.matmul(
                        ps_s[:],
                        WSv(mc, kc),
                        hsT_v[:, kc, :],
                        start=(kc == 0),
                        stop=False,
                    )
                # + (b_S + b_V) broadcast along packed cols: rank-1 K=1 matmul
                nc.tensor.matmul(
                    ps_s[:],
                    bSV_sb[0:1, mc * 128 : (mc + 1) * 128],
                    onesP[0:1, :],
                    start=False,
                    stop=True,
                )
                # ST_dup[d, mc, p, 2] <- ps_s duplicated over pair axis; for
                # chunk 0 split slot 0 out so the head of the ACT chain isn't
                # gated on the full-P cast
                if mc == 0:
                    nc.vector.tensor_copy(
                        ST_dup[:, mc, 0 : offs[1], :],
                        ps_s[:, 0 : offs[1]]
                        .unsqueeze(2)
                        .broadcast_to([128, bounds[0], 2]),
                    )
                    nc.vector.tensor_copy(
                        ST_dup[:, mc, offs[1] :, :],
                        ps_s[:, offs[1] :]
                        .unsqueeze(2)
                        .broadcast_to([128, P - offs[1], 2]),
                    )
                else:
                    nc.vector.tensor_copy(
                        ST_dup[:, mc, :, :],
                        ps_s[:].unsqueeze(2).broadcast_to([128, P, 2]),
                    )

            def proj_V0(mc):
                # V chunk for slot 0 only (gated just on hvT0 + WV_mc)
                pv_t = pvp.tile([128, BPC, 128], f32, tag="pv")
                for kc in range(NC):
                    nc.tensor.matmul(
                        pv_t[:, 0, :],
                        WVv(mc, kc),
                        hvT0_sb[:, kc, :],
                        start=(kc == 0),
                        stop=(kc == NC - 1),
                    )
                nc.vector.tensor_copy(VT_sb[:, 0, mc, :], pv_t[:, 0, :])

            def proj_Vr(mc):
                # V chunk for slots 1..3, batched in the rhs free dim
                pv_t = pvp.tile([128, BPC, 128], f32, tag="pv")
                for kc in range(NC):
                    nc.tensor.matmul(
                        pv_t[:, 1:BPC, :],
                        WVv(mc, kc),
                        hvTr_sb[:, :, kc, :],
                        start=(kc == 0),
                        stop=(kc == NC - 1),
                    )
                nc.vector.tensor_copy(VT_sb[:, 1:BPC, mc, :], pv_t[:, 1:BPC, :])

            def ep_add(k, c, ep_slice, n0, n1):
                """e_pre = VT (+bcast over n) + ST' (+bcast over t-pairs)
                for slot k chunk c, rows [n0,n1), into ep_slice."""
                nn = n1 - n0
                nc.vector.tensor_add(
                    ep_slice.rearrange("p n (t two) -> p n t two", two=2),
                    VT_sb[:, k, c, :]
                    .rearrange("p (t two) -> p t two", two=2)
                    .unsqueeze(1)
                    .broadcast_to([128, nn, 64, 2]),
                    ST_dup[:, c, offs[k] + n0 : offs[k] + n1, :]
                    .unsqueeze(2)
                    .broadcast_to([128, nn, 64, 2]),
                )

            def beta_mms(k, c, eb, beta_acc, n0, n1):
                # all 4 chunks accumulate into one [128, bound] PSUM plane:
                # the slot's first matmul (c0,n0) start=True clears the bank's
                # has_written bits; every later matmul accumulates (c>0, bits
                # set) or overwrites (c==0, bits clear) per element
                bk = bounds[k]
                for n in range(n0, n1):
                    nc.tensor.matmul(
                        beta_acc[:, n : n + 1],
                        eb[:, c, n, :],
                        Ww_v[:, c : c + 1],
                        start=(c == 0 and n == 0),
                        stop=(c == NC - 1 and n == bk - 1),
                    )

            def add_tanh(k, c, eb, halve=False):
                """add (DVE 2x) -> tanh (ACT) for one chunk; betas deferred.
                halve=True splits by n-halves so the tanh starts on the
                first half's add (closes the early-stream supply bubble)."""
                bk = bounds[k]
                ep = eprep.tile([128, B0, 128], bf16, tag="ep")
                ranges = ((0, max(1, bk // 2)), (max(1, bk // 2), bk)) if halve else ((0, bk),)
                for n0, n1 in ranges:
                    if n1 <= n0:
                        continue
                    ep_add(k, c, ep[:, n0:n1, :], n0, n1)
                    nc.scalar.activation(
                        eb[:, c, n0:n1, :],
                        ep[:, n0:n1, :],
                        mybir.ActivationFunctionType.Tanh,
                    )

            def add_tanh_head(k, c, eb, beta_big):
                """First granule: add/tanh split on an accelerating n-grid so
                the ACT chain starts on a tiny first dependency while each
                later add still finishes before the previous tanh ends."""
                bk = bounds[k]
                cuts = sorted({0, min(4, bk), min(12, bk), min(21, bk), bk})
                ep = eprep.tile([128, B0, 128], bf16, tag="ep")
                for n0, n1 in zip(cuts[:-1], cuts[1:]):
                    if n1 <= n0:
                        continue
                    ep_add(k, c, ep[:, n0:n1, :], n0, n1)
                    nc.scalar.activation(
                        eb[:, c, n0:n1, :],
                        ep[:, n0:n1, :],
                        mybir.ActivationFunctionType.Tanh,
                    )

            def granule2(k, c0, eb, beta_big, tail=False):
                """2-chunk granule: two adds, one tanh, betas for both chunks.
                tail=True splits the second chunk's tanh by n-halves so the
                final beta matmuls and softmax start earlier."""
                bk = bounds[k]
                ep = eprep2.tile([128, 2, bounds[1], 128], bf16, tag="ep2")
                ep_add(k, c0, ep[:, 0, 0:bk, :], 0, bk)
                ep_add(k, c0 + 1, ep[:, 1, 0:bk, :], 0, bk)
                if not tail:
                    nc.scalar.activation(
                        eb[:, c0 : c0 + 2, :, :],
                        ep[:, :, 0:bk, :],
                        mybir.ActivationFunctionType.Tanh,
                    )
                    beta_mms(k, c0, eb, beta_big, 0, bk)
                    beta_mms(k, c0 + 1, eb, beta_big, 0, bk)
                else:
                    h = max(1, bk // 2)
                    nc.scalar.activation(
                        eb[:, c0, :, :],
                        ep[:, 0, 0:bk, :],
                        mybir.ActivationFunctionType.Tanh,
                    )
                    beta_mms(k, c0, eb, beta_big, 0, bk)
                    nc.scalar.activation(
                        eb[:, c0 + 1, 0:h, :],
                        ep[:, 1, 0:h, :],
                        mybir.ActivationFunctionType.Tanh,
                    )
                    beta_mms(k, c0 + 1, eb, beta_big, 0, h)
                    if h < bk:
                        nc.scalar.activation(
                            eb[:, c0 + 1, h:bk, :],
                            ep[:, 1, h:bk, :],
                            mybir.ActivationFunctionType.Tanh,
                        )
                        beta_mms(k, c0 + 1, eb, beta_big, h, bk)

            def softmax_final(k):
                bk = bounds[k]
                beta_acc = slot_tiles[k][1]
                qa = softp.tile([128, B0], f32, tag="qa")
                # + bwm: b_w on valid cols, -50 on pad cols (exp kills pads)
                nc.vector.tensor_add(
                    qa[:, 0:bk],
                    beta_acc[:, 0:bk],
                    bwm_v[:, offs[k] : offs[k] + bk],
                )
                t1 = softp.tile([128, B0], f32, tag="t1")
                nc.scalar.activation(
                    t1[:, 0:bk], qa[:, 0:bk], mybir.ActivationFunctionType.Exp
                )
                Qs = softp.tile([128, 1], f32, tag="Z1")
                nc.vector.tensor_reduce(
                    Qs[:], t1[:, 0:bk], mybir.AxisListType.X, mybir.AluOpType.add
                )
                recip = softp.tile([128, 1], f32, tag="recip")
                nc.vector.reciprocal(recip[:], Qs[:])
                # ---- out[k] = (t1 @ h_s_masked[k]) * recip ----
                qT_ps = pqtp.tile([B0, 128], f32, tag="qt")
                nc.tensor.transpose(qT_ps[0:bk, :], t1[:, 0:bk], ident[:])
                qT = softp.tile([B0, 128], bf16, tag="qTs")
                nc.vector.tensor_copy(qT[0:bk, :], qT_ps[0:bk, :])
                out_ps = pfinp.tile([128, D], f32, tag="out")
                out_sb = outp.tile([128, D], bf16, tag="osb")
                nc.tensor.matmul(
                    out_ps[:], qT[0:bk, :], hs_sb[0:bk, k, :], start=True, stop=True
                )
                if k == BPC - 1:
                    # tail: ACT is idle by now and reads PSUM faster than DVE
                    nc.scalar.activation(
                        out_sb[:],
                        out_ps[:],
                        mybir.ActivationFunctionType.Copy,
                        scale=recip[:],
                    )
                else:
                    nc.vector.tensor_scalar_mul(out_sb[:], out_ps[:], recip[:])
                nc.sync.dma_start(out=out_d[k], in_=out_sb[:])

            # ---- slot 0 interleaved with projections; beta matmuls are
            # emitted one chunk late so the PE queue never blocks a
            # projection behind tanh-gated work ----
            slot_tiles = {}

            def alloc_slot(k):
                slot_tiles[k] = (
                    ebigp.tile(
                        [128, NC, bounds[k], 128], bf16, tag=f"e{k}", name=f"eb{k}"
                    ),
                    pbetap.tile([128, bounds[k]], f32, tag="beta", name=f"bb{k}"),
                )

            for mc in range(NC):
                proj_S(mc)
                proj_V0(mc)
                if mc == 0:
                    alloc_slot(0)
                    add_tanh_head(0, 0, *slot_tiles[0])
                else:
                    beta_mms(0, mc - 1, *slot_tiles[0], 0, bounds[0])
                    add_tanh(0, mc, slot_tiles[0][0], halve=(mc == 1))
            for mc in range(NC):
                proj_Vr(mc)
            beta_mms(0, NC - 1, *slot_tiles[0], 0, bounds[0])

            # ---- remaining slots, softmax/final pipelined one slot late ----
            for k in range(1, BPC):
                alloc_slot(k)
                eb, beta_big = slot_tiles[k]
                last = k == BPC - 1
                if not last:
                    granule2(k, 0, eb, beta_big)
                    softmax_final(k - 1)
                    granule2(k, 2, eb, beta_big)
                else:
                    # last slot: softmax first so its DVE ops don't sit
                    # between this slot's adds (the end is the critical tail)
                    softmax_final(k - 1)
                    granule2(k, 0, eb, beta_big)
                    granule2(k, 2, eb, beta_big, tail=True)
            softmax_final(BPC - 1)

    nc.compile()
    return nc


def _get_nc(bounds):
    key = tuple(bounds)
    if key not in _CACHE:
        _CACHE[key] = _build(list(bounds))
    return _CACHE[key]


def _plan(lengths):
    """Sort batches by length desc; slot k on core c <- sorted rank 8k+c.
    Returns (order, bounds)."""
    lengths = np.asarray(lengths).reshape(-1)
    order = np.argsort(-lengths, kind="stable")
    bounds = [int(lengths[order[NCORES * k]]) for k in range(BPC)]
    return order, bounds


def _make_in_maps(order, bounds, h_s, h_v, lengths, W_S, b_S, W_V, b_V, W_w, b_w):
    f32 = np.float32
    h_s = np.asarray(h_s, dtype=f32)
    h_v = np.asarray(h_v, dtype=f32)
    lengths = np.asarray(lengths).reshape(-1)
    offs = np.concatenate([[0], np.cumsum(bounds)]).astype(int)
    P = int(offs[-1])
    B0 = bounds[0]

    # weights, chunked + cast once (shared across cores); mc outermost
    WS = np.ascontiguousarray(
        np.asarray(W_S, f32).reshape(NC, 128, NC, 128).transpose(2, 1, 0, 3)
    )  # [mc, p, kc, 128]
    WV = np.ascontiguousarray(
        np.asarray(W_V, f32).reshape(NC, 128, NC, 128).transpose(2, 1, 0, 3)
    )
    Ww = np.ascontiguousarray(np.asarray(W_w, f32).reshape(NC, 128).T)  # [128, NC]
    bSV = (np.asarray(b_S, f32) + np.asarray(b_V, f32)).reshape(1, D)
    bw_val = f32(np.asarray(b_w).reshape(-1)[0])

    try:
        import ml_dtypes

        bf16 = ml_dtypes.bfloat16
    except ImportError:
        import jax.numpy as jnp

        bf16 = jnp.bfloat16

    def to_bf16(x):
        return np.asarray(x, dtype=bf16)

    WS_b = to_bf16(WS)
    WV_b = to_bf16(WV)
    Ww_b = to_bf16(Ww)
    bSV_b = to_bf16(bSV)
    WSr_b = np.ascontiguousarray(WS_b[1:])
    WVr_b = np.ascontiguousarray(WV_b[1:])

    in_maps = []
    for core in range(NCORES):
        batches = [int(order[NCORES * k + core]) for k in range(BPC)]
        hv_c = h_v[batches]  # (BPC, T, D)
        hvT = np.ascontiguousarray(
            hv_c.reshape(BPC, T, NC, 128).transpose(0, 3, 2, 1)
        )  # (slot, 128p, kc, t)
        hvT_b = to_bf16(hvT)
        hsT = np.zeros((128, NC, P), dtype=f32)
        hs_r = np.zeros((B0, BPC, D), dtype=f32)  # (n, slot, D), masked rows 0
        bwm = np.full((128, P), -50.0, dtype=f32)
        for k, b in enumerate(batches):
            L = int(lengths[b])
            bk = bounds[k]
            Lk = min(L, bk)
            hk = h_s[b, :Lk]  # (Lk, D)
            hsT[:, :, offs[k] : offs[k] + Lk] = hk.reshape(Lk, NC, 128).transpose(
                2, 1, 0
            )
            hs_r[:Lk, k, :] = hk
            bwm[:, offs[k] : offs[k] + Lk] = bw_val
        # blobs: A1 = hsT | WS0 ; A3 = WV0 | Ww | bwm   (all bf16)
        A1 = np.concatenate(
            [to_bf16(hsT).reshape(128, NC * P), WS_b[0].reshape(128, NC * 128)],
            axis=1,
        )
        A3 = np.concatenate(
            [WV_b[0].reshape(128, NC * 128), Ww_b, to_bf16(bwm)], axis=1
        )
        in_maps.append(
            {
                "A1": np.ascontiguousarray(A1),
                "hvT0": np.ascontiguousarray(hvT_b[0]),
                "A3": np.ascontiguousarray(A3),
                "WSr": WSr_b,
                "WVr": WVr_b,
                "hvTr": np.ascontiguousarray(hvT_b[1:].transpose(1, 0, 2, 3)),
                "bSV": bSV_b,
                "hs": to_bf16(hs_r),
            }
        )
    return in_maps


def run(inputs: dict, trace: bool = False):
    """Run on 8 NeuronCores; returns (output, BassKernelResults)."""
    from concourse import bass_utils

    order, bounds = _plan(inputs["lengths"])
    nc = _get_nc(bounds)
    in_maps = _make_in_maps(order, bounds, **inputs)
    res = bass_utils.run_bass_kernel_spmd(
        nc, in_maps, core_ids=list(range(NCORES)), trace=trace
    )
    full = np.zeros((B, T, D), dtype=np.float32)
    for core in range(NCORES):
        o = np.asarray(res.results[core]["out"], dtype=np.float32)
        for k in range(BPC):
            full[int(order[NCORES * k + core])] = o[k]
    return full, res


def kernel(**inputs) -> np.ndarray:
    out, _ = run(inputs, trace=False)
    return out


# revision 35
# speedup vs baseline: 1.2120x; 1.0314x over previous
"""Trainium2 Bass kernel for InteractorwoLSTM additive attention.

out[b,t,:] = alpha[b,t,:] @ h_s[b]  with
  beta[b,t,n] = W_w . tanh(h_s[b,n]@W_S + b_S + h_v[b,t]@W_V + b_V) + b_w
  alpha = masked-softmax(beta) per reference semantics.

Design: length-truncated slots + blob DMA + PSUM-accumulated beta.
Measured ~54us on HW (baseline 82.4us); ACT tanh stream runs at ~100%
occupancy, so the kernel sits on the ACT roofline for its element count.

Positions n >= lengths[b] never influence the output (the reference's
masked-softmax renormalization cancels them), so the tanh/add/beta work
for those positions is skipped. lengths are known on the host before
compile, so the program is built for the actual length profile:
batches sorted by length desc; slot k on core c holds sorted rank
8k+c; slot k's n-bound = L(rank 8k) (SPMD shares one program). For the
seed-0 data: bounds [30,17,12,8] -> 67 packed n-columns vs 120
(~44% less work on ACT -- the bottleneck engine at 1 elem/cycle/lane).

Ramp design (the first ~8us are framework preamble + DMA):
  - descriptor generation costs ~0.65us per dma_start, serialized on
    the ISSUING sequencer (and modeled so by the tile scheduler's
    CoreSim -- a dma_start on the scalar queue serializes against ACT
    dispatch and makes the scheduler statically sink dependent work).
    So: no input DMA ever rides the scalar queue; S-side pieces ride
    sync, V-side pieces ride gpsimd, first-granule deps first; the
    first pieces are host-packed blobs (A1 = hsT|WS0, A3 = WV0|Ww|bwm)
    with 1.5-3.9KB contiguous per-partition runs.
  - slot-0 V projections are per-chunk so the tanh cadence never waits
    for other slots' hvT; slots 1-3 V projections batched+deferred.
  - slot-0 beta matmuls are emitted one chunk late so the PE queue
    never blocks a projection behind tanh-gated work.
  - b_w and the pad-kill fold into one per-column bias (bwm: b_w on
    valid cols, -50 on pads -> exp underflows, no mask mult).
  - beta accumulates across all 4 D-chunks into one [128, bound] PSUM
    plane (slot's first matmul start=True clears the bank; the rest
    accumulate per-element via has_written), so the softmax reads one
    qa = beta + bwm add straight from PSUM.
  - last granule's tanh splits by n-halves; the last out-scale runs on
    ACT (idle at the tail, faster PSUM reads); dummy tanh at t=0 pulls
    the ACT table load under the input DMAs.
"""

import numpy as np

B, T, N = 32, 128, 30
D = 512
NCORES = 8
BPC = B // NCORES  # batch slots per core
NC = D // 128  # 4 chunks of 128 along D

_CACHE = {}


def _build(bounds):
    import concourse.bacc as bacc
    import concourse.tile as tile
    from concourse import mybir
    from concourse.masks import make_identity

    f32 = mybir.dt.float32
    bf16 = mybir.dt.bfloat16

    offs = [0]
    for b in bounds:
        offs.append(offs[-1] + b)
    P = offs[-1]
    B0 = bounds[0]
    # blob column layouts (bf16, [128, X])
    XA1 = NC * P + NC * 128  # hsT | WS0
    XA3 = NC * 128 + NC + P  # WV0 | Ww | bwm

    nc = bacc.Bacc(
        "TRN2",
        target_bir_lowering=False,
        debug=False,
        enable_asserts=False,
        num_devices=NCORES,
    )

    # ---- DRAM I/O (host-packed blobs; see _make_in_maps) ----
    A1_d = nc.dram_tensor("A1", [128, XA1], bf16, kind="ExternalInput").ap()
    hvT0_d = nc.dram_tensor("hvT0", [128, NC, 128], bf16, kind="ExternalInput").ap()
    A3_d = nc.dram_tensor("A3", [128, XA3], bf16, kind="ExternalInput").ap()
    WSr_d = nc.dram_tensor("WSr", [NC - 1, 128, NC, 128], bf16, kind="ExternalInput").ap()
    WVr_d = nc.dram_tensor("WVr", [NC - 1, 128, NC, 128], bf16, kind="ExternalInput").ap()
    hvTr_d = nc.dram_tensor("hvTr", [128, BPC - 1, NC, 128], bf16, kind="ExternalInput").ap()
    bSV_d = nc.dram_tensor("bSV", [1, D], bf16, kind="ExternalInput").ap()
    hs_d = nc.dram_tensor("hs", [B0, BPC, D], bf16, kind="ExternalInput").ap()
    out_d = nc.dram_tensor("out", [BPC, T, D], bf16, kind="ExternalOutput").ap()

    with tile.TileContext(nc) as tc:
        with (
            tc.tile_pool(name="const", bufs=1) as const,
            tc.tile_pool(name="epre", bufs=3) as eprep,
            tc.tile_pool(name="epre2", bufs=3) as eprep2,
            tc.tile_pool(name="ebig", bufs=1) as ebigp,
            tc.tile_pool(name="soft", bufs=3) as softp,
            tc.tile_pool(name="outp", bufs=3) as outp,
            tc.tile_pool(name="pv", bufs=2, space="PSUM") as pvp,
            tc.tile_pool(name="ps", bufs=2, space="PSUM") as psp,
            tc.tile_pool(name="pbeta", bufs=2, space="PSUM") as pbetap,
            tc.tile_pool(name="pqt", bufs=1, space="PSUM") as pqtp,
            tc.tile_pool(name="pfin", bufs=1, space="PSUM") as pfinp,
        ):
            # ---- force the exp/tanh ACT table load to start at t=0 so it
            # hides under the input DMAs (the set covers tanh+exp+copy)
            warm = const.tile([1, 2], f32)
            nc.vector.memset(warm[:], 0.0)
            warm2 = const.tile([1, 2], f32)
            nc.scalar.activation(warm2[:], warm[:], mybir.ActivationFunctionType.Tanh)

            A1_sb = const.tile([128, XA1], bf16)
            hvT0_sb = const.tile([128, NC, 128], bf16)
            A3_sb = const.tile([128, XA3], bf16)
            WSr_sb = const.tile([128, NC - 1, NC, 128], bf16)
            WVr_sb = const.tile([128, NC - 1, NC, 128], bf16)
            hvTr_sb = const.tile([128, BPC - 1, NC, 128], bf16)
            bSV_sb = const.tile([1, D], bf16)
            hs_sb = const.tile([B0, BPC, D], bf16)

            # dma_start occupies its ISSUING engine (~0.5-0.7us descriptor
            # gen, both in the scheduler's CoreSim and on the real
            # sequencer). The scalar queue shares the ACT dispatch path, so
            # input DMAs must NEVER ride it -- S-side pieces go on sync
            # (SP is otherwise idle), V-side pieces on gpsimd, each queue
            # in consumption order with the first-granule deps leading.
            nc.sync.dma_start(out=A1_sb[:], in_=A1_d)
            nc.gpsimd.dma_start(out=hvT0_sb[:], in_=hvT0_d)
            nc.sync.dma_start(out=bSV_sb[:], in_=bSV_d)
            # A3 rides the scalar queue: its only input DMA, issued after
            # the warm ops drain (~8.5us), so the two V-side transfers
            # (hvT0 on gpsimd, A3 here) run in parallel instead of
            # serializing behind one sequencer's descriptor gens
            nc.scalar.dma_start(out=A3_sb[:], in_=A3_d)
            nc.sync.dma_start(out=WSr_sb[:, 0, :, :], in_=WSr_d[0])
            nc.gpsimd.dma_start(out=WVr_sb[:, 0, :, :], in_=WVr_d[0])
            nc.sync.dma_start(out=WSr_sb[:, 1, :, :], in_=WSr_d[1])
            nc.gpsimd.dma_start(out=WVr_sb[:, 1, :, :], in_=WVr_d[1])
            nc.sync.dma_start(out=WSr_sb[:, 2, :, :], in_=WSr_d[2])
            nc.gpsimd.dma_start(out=WVr_sb[:, 2, :, :], in_=WVr_d[2])
            nc.sync.dma_start(out=hvTr_sb[:], in_=hvTr_d)
            nc.sync.dma_start(out=hs_sb[:], in_=hs_d)

            # views into the blobs
            hsT_v = A1_sb[:, 0 : NC * P].rearrange("p (k x) -> p k x", k=NC)
            WS0_v = A1_sb[:, NC * P :].rearrange("p (k x) -> p k x", k=NC)
            WV0_v = A3_sb[:, 0 : NC * 128].rearrange("p (k x) -> p k x", k=NC)
            Ww_v = A3_sb[:, NC * 128 : NC * 128 + NC]
            bwm_v = A3_sb[:, NC * 128 + NC :]

            def WSv(mc, kc):
                return WS0_v[:, kc, :] if mc == 0 else WSr_sb[:, mc - 1, kc, :]

            def WVv(mc, kc):
                return WV0_v[:, kc, :] if mc == 0 else WVr_sb[:, mc - 1, kc, :]

            ident = const.tile([128, 128], f32)
            make_identity(nc, ident[:])
            onesP = const.tile([1, P], bf16)
            nc.vector.memset(onesP[:], 1.0)

            VT_sb = const.tile([128, BPC, NC, 128], bf16)
            ST_dup = const.tile([128, NC, P, 2], bf16)

            # ---- helpers -------------------------------------------------
            def proj_S(mc):
                # S chunk: ST'[d, packed(slot, n)] for all slots
                ps_s = psp.tile([128, P], f32, tag="ps")
                for kc in range(NC):
                    nc.tensor.matmul(
                        ps_s[:],
                        WSv(mc, kc),
                        hsT_v[:, kc, :],
                        start=(kc == 0),
                        stop=False,
                    )
                # + (b_S + b_V) broadcast along packed cols: rank-1 K=1 matmul
                nc.tensor.matmul(
                    ps_s[:],
                    bSV_sb[0:1, mc * 128 : (mc + 1) * 128],
                    onesP[0:1, :],
                    start=False,
                    stop=True,
                )
                # ST_dup[d, mc, p, 2] <- ps_s duplicated over pair axis; for
                # chunk 0 split slot 0 out so the head of the ACT chain isn't
                # gated on the full-P cast
                if mc == 0:
                    nc.vector.tensor_copy(
                        ST_dup[:, mc, 0 : offs[1], :],
                        ps_s[:, 0 : offs[1]]
                        .unsqueeze(2)
                        .broadcast_to([128, bounds[0], 2]),
                    )
                    nc.vector.tensor_copy(
                        ST_dup[:, mc, offs[1] :, :],
                        ps_s[:, offs[1] :]
                        .unsqueeze(2)
                        .broadcast_to([128, P - offs[1], 2]),
                    )
                else:
                    nc.vector.tensor_copy(
                        ST_dup[:, mc, :, :],
                        ps_s[:].unsqueeze(2).broadcast_to([128, P, 2]),
                    )

            def proj_V0(mc):
                # V chunk for slot 0 only (gated just on hvT0 + WV_mc)
                pv_t = pvp.tile([128, BPC, 128], f32, tag="pv")
                for kc in range(NC):
                    nc.tensor.matmul(
                        pv_t[:, 0, :],
                        WVv(mc, kc),
                        hvT0_sb[:, kc, :],
                        start=(kc == 0),
                        stop=(kc == NC - 1),
                    )
                nc.vector.tensor_copy(VT_sb[:, 0, mc, :], pv_t[:, 0, :])

            def proj_Vr(mc):
                # V chunk for slots 1..3, batched in the rhs free dim
                pv_t = pvp.tile([128, BPC, 128], f32, tag="pv")
                for kc in range(NC):
                    nc.tensor.matmul(
                        pv_t[:, 1:BPC, :],
                        WVv(mc, kc),
                        hvTr_sb[:, :, kc, :],
                        start=(kc == 0),
                        stop=(kc == NC - 1),
                    )
                nc.vector.tensor_copy(VT_sb[:, 1:BPC, mc, :], pv_t[:, 1:BPC, :])

            def ep_add(k, c, ep_slice, n0, n1):
                """e_pre = VT (+bcast over n) + ST' (+bcast over t-pairs)
                for slot k chunk c, rows [n0,n1), into ep_slice."""
                nn = n1 - n0
                nc.vector.tensor_add(
                    ep_slice.rearrange("p n (t two) -> p n t two", two=2),
                    VT_sb[:, k, c, :]
                    .rearrange("p (t two) -> p t two", two=2)
                    .unsqueeze(1)
                    .broadcast_to([128, nn, 64, 2]),
                    ST_dup[:, c, offs[k] + n0 : offs[k] + n1, :]
                    .unsqueeze(2)
                    .broadcast_to([128, nn, 64, 2]),
                )

            def beta_mms(k, c, eb, beta_acc, n0, n1):
                # all 4 chunks accumulate into one [128, bound] PSUM plane:
                # the slot's first matmul (c0,n0) start=True clears the bank's
                # has_written bits; every later matmul accumulates (c>0, bits
                # set) or overwrites (c==0, bits clear) per element
                bk = bounds[k]
                for n in range(n0, n1):
                    nc.tensor.matmul(
                        beta_acc[:, n : n + 1],
                        eb[:, c, n, :],
                        Ww_v[:, c : c + 1],
                        start=(c == 0 and n == 0),
                        stop=(c == NC - 1 and n == bk - 1),
                    )

            def add_tanh(k, c, eb, halve=False):
                """add (DVE 2x) -> tanh (ACT) for one chunk; betas deferred.
                halve=True splits by n-halves so the tanh starts on the
                first half's add (closes the early-stream supply bubble)."""
                bk = bounds[k]
                ep = eprep.tile([128, B0, 128], bf16, tag="ep")
                ranges = ((0, max(1, bk // 2)), (max(1, bk // 2), bk)) if halve else ((0, bk),)
                for n0, n1 in ranges:
                    if n1 <= n0:
                        continue
                    ep_add(k, c, ep[:, n0:n1, :], n0, n1)
                    nc.scalar.activation(
                        eb[:, c, n0:n1, :],
                        ep[:, n0:n1, :],
                        mybir.ActivationFunctionType.Tanh,
                    )

            def add_tanh_head(k, c, eb, beta_big):
                """First granule: add/tanh split on an accelerating n-grid so
                the ACT chain starts on a tiny first dependency while each
                later add still finishes before the previous tanh ends."""
                bk = bounds[k]
                cuts = sorted({0, min(4, bk), min(12, bk), min(21, bk), bk})
                ep = eprep.tile([128, B0, 128], bf16, tag="ep")
                for n0, n1 in zip(cuts[:-1], cuts[1:]):
                    if n1 <= n0:
                        continue
                    ep_add(k, c, ep[:, n0:n1, :], n0, n1)
                    nc.scalar.activation(
                        eb[:, c, n0:n1, :],
                        ep[:, n0:n1, :],
                        mybir.ActivationFunctionType.Tanh,
                    )

            def granule2(k, c0, eb, beta_big, tail=False):
                """2-chunk granule: two adds, one tanh, betas for both chunks.
                tail=True splits the second chunk's tanh by n-halves so the
                final beta matmuls and softmax start earlier."""
                bk = bounds[k]
                ep = eprep2.tile([128, 2, bounds[1], 128], bf16, tag="ep2")
                ep_add(k, c0, ep[:, 0, 0:bk, :], 0, bk)
                ep_add(k, c0 + 1, ep[:, 1, 0:bk, :], 0, bk)
                if not tail:
                    nc.scalar.activation(
                        eb[:, c0 : c0 + 2, :, :],
                        ep[:, :, 0:bk, :],
                        mybir.ActivationFunctionType.Tanh,
                    )
                    beta_mms(k, c0, eb, beta_big, 0, bk)
                    beta_mms(k, c0 + 1, eb, beta_big, 0, bk)
                else:
                    h = max(1, bk // 2)
                    nc.scalar.activation(
                        eb[:, c0, :, :],
                        ep[:, 0, 0:bk, :],
                        mybir.ActivationFunctionType.Tanh,
                    )
                    beta_mms(k, c0, eb, beta_big, 0, bk)
                    nc.scalar.activation(
                        eb[:, c0 + 1, 0:h, :],
                        ep[:, 1, 0:h, :],
                        mybir.ActivationFunctionType.Tanh,
                    )
                    beta_mms(k, c0 + 1, eb, beta_big, 0, h)
                    if h < bk:
                        nc.scalar.activation(
                            eb[:, c0 + 1, h:bk, :],
                            ep[:, 1, h:bk, :],
                            mybir.ActivationFunctionType.Tanh,
                        )
                        beta_mms(k, c0 + 1, eb, beta_big, h, bk)

            def softmax_final(k):
                bk = bounds[k]
                beta_acc = slot_tiles[k][1]
                qa = softp.tile([128, B0], f32, tag="qa")
                # + bwm: b_w on valid cols, -50 on pad cols (exp kills pads)
                nc.vector.tensor_add(
                    qa[:, 0:bk],
                    beta_acc[:, 0:bk],
                    bwm_v[:, offs[k] : offs[k] + bk],
                )
                t1 = softp.tile([128, B0], f32, tag="t1")
                nc.scalar.activation(
                    t1[:, 0:bk], qa[:, 0:bk], mybir.ActivationFunctionType.Exp
                )
                Qs = softp.tile([128, 1], f32, tag="Z1")
                nc.vector.tensor_reduce(
                    Qs[:], t1[:, 0:bk], mybir.AxisListType.X, mybir.AluOpType.add
                )
                recip = softp.tile([128, 1], f32, tag="recip")
                nc.vector.reciprocal(recip[:], Qs[:])
                # ---- out[k] = (t1 @ h_s_masked[k]) * recip ----
                qT_ps = pqtp.tile([B0, 128], f32, tag="qt")
                nc.tensor.transpose(qT_ps[0:bk, :], t1[:, 0:bk], ident[:])
                qT = softp.tile([B0, 128], bf16, tag="qTs")
                nc.vector.tensor_copy(qT[0:bk, :], qT_ps[0:bk, :])
                out_ps = pfinp.tile([128, D], f32, tag="out")
                out_sb = outp.tile([128, D], bf16, tag="osb")
                nc.tensor.matmul(
                    out_ps[:], qT[0:bk, :], hs_sb[0:bk, k, :], start=True, stop=True
                )
                if k == BPC - 1:
                    # tail: ACT is idle by now and reads PSUM faster than DVE
                    nc.scalar.activation(
                        out_sb[:],
                        out_ps[:],
                        mybir.ActivationFunctionType.Copy,
                        scale=recip[:],
                    )
                else:
                    nc.vector.tensor_scalar_mul(out_sb[:], out_ps[:], recip[:])
                nc.sync.dma_start(out=out_d[k], in_=out_sb[:])

            # ---- slot 0 interleaved with projections; beta matmuls are
            # emitted one chunk late so the PE queue never blocks a
            # projection behind tanh-gated work ----
            slot_tiles = {}

            def alloc_slot(k):
                slot_tiles[k] = (
                    ebigp.tile(
                        [128, NC, bounds[k], 128], bf16, tag=f"e{k}", name=f"eb{k}"
                    ),
                    pbetap.tile([128, bounds[k]], f32, tag="beta", name=f"bb{k}"),
                )

            for mc in range(NC):
                proj_S(mc)
                proj_V0(mc)
                if mc == 0:
                    alloc_slot(0)
                    add_tanh_head(0, 0, *slot_tiles[0])
                else:
                    beta_mms(0, mc - 1, *slot_tiles[0], 0, bounds[0])
                    add_tanh(0, mc, slot_tiles[0][0], halve=(mc == 1))
            for mc in range(NC):
                proj_Vr(mc)
            beta_mms(0, NC - 1, *slot_tiles[0], 0, bounds[0])

            # ---- remaining slots, softmax/final pipelined one slot late ----
            for k in range(1, BPC):
                alloc_slot(k)
                eb, beta_big = slot_tiles[k]
                last = k == BPC - 1
                if not last:
                    granule2(k, 0, eb, beta_big)
                    softmax_final(k - 1)
                    granule2(k, 2, eb, beta_big)
                else:
                    # last slot: softmax first so its DVE ops don't sit
                    # between this slot's adds (the end is the critical tail)
                    softmax_final(k - 1)
                    granule2(k, 0, eb, beta_big)
                    granule2(k, 2, eb, beta_big, tail=True)
            softmax_final(BPC - 1)

    nc.compile()
    return nc


def _get_nc(bounds):
    key = tuple(bounds)
    if key not in _CACHE:
        _CACHE[key] = _build(list(bounds))
    return _CACHE[key]


def _plan(lengths):
    """Sort batches by length desc; slot k on core c <- sorted rank 8k+c.
    Returns (order, bounds)."""
    lengths = np.asarray(lengths).reshape(-1)
    order = np.argsort(-lengths, kind="stable")
    bounds = [int(lengths[order[NCORES * k]]) for k in range(BPC)]
    return order, bounds


def _make_in_maps(order, bounds, h_s, h_v, lengths, W_S, b_S, W_V, b_V, W_w, b_w):
    f32 = np.float32
    h_s = np.asarray(h_s, dtype=f32)
    h_v = np.asarray(h_v, dtype=f32)
    lengths = np.asarray(lengths).reshape(-1)
    offs = np.concatenate([[0], np.cumsum(bounds)]).astype(int)
    P = int(offs[-1])
    B0 = bounds[0]

    # weights, chunked + cast once (shared across cores); mc outermost
    WS = np.ascontiguousarray(
        np.asarray(W_S, f32).reshape(NC, 128, NC, 128).transpose(2, 1, 0, 3)
    )  # [mc, p, kc, 128]
    WV = np.ascontiguousarray(
        np.asarray(W_V, f32).reshape(NC, 128, NC, 128).transpose(2, 1, 0, 3)
    )
    Ww = np.ascontiguousarray(np.asarray(W_w, f32).reshape(NC, 128).T)  # [128, NC]
    bSV = (np.asarray(b_S, f32) + np.asarray(b_V, f32)).reshape(1, D)
    bw_val = f32(np.asarray(b_w).reshape(-1)[0])

    try:
        import ml_dtypes

        bf16 = ml_dtypes.bfloat16
    except ImportError:
        import jax.numpy as jnp

        bf16 = jnp.bfloat16

    def to_bf16(x):
        return np.asarray(x, dtype=bf16)

    WS_b = to_bf16(WS)
    WV_b = to_bf16(WV)
    Ww_b = to_bf16(Ww)
    bSV_b = to_bf16(bSV)
    WSr_b = np.ascontiguousarray(WS_b[1:])
    WVr_b = np.ascontiguousarray(WV_b[1:])

    in_maps = []
    for core in range(NCORES):
        batches = [int(order[NCORES * k + core]) for k in range(BPC)]
        hv_c = h_v[batches]  # (BPC, T, D)
        hvT = np.ascontiguousarray(
            hv_c.reshape(BPC, T, NC, 128).transpose(0, 3, 2, 1)
        )  # (slot, 128p, kc, t)
        hvT_b = to_bf16(hvT)
        hsT = np.zeros((128, NC, P), dtype=f32)
        hs_r = np.zeros((B0, BPC, D), dtype=f32)  # (n, slot, D), masked rows 0
        bwm = np.full((128, P), -50.0, dtype=f32)
        for k, b in enumerate(batches):
            L = int(lengths[b])
            bk = bounds[k]
            Lk = min(L, bk)
            hk = h_s[b, :Lk]  # (Lk, D)
            hsT[:, :, offs[k] : offs[k] + Lk] = hk.reshape(Lk, NC, 128).transpose(
                2, 1, 0
            )
            hs_r[:Lk, k, :] = hk
            bwm[:, offs[k] : offs[k] + Lk] = bw_val
        # blobs: A1 = hsT | WS0 ; A3 = WV0 | Ww | bwm   (all bf16)
        A1 = np.concatenate(
            [to_bf16(hsT).reshape(128, NC * P), WS_b[0].reshape(128, NC * 128)],
            axis=1,
        )
        A3 = np.concatenate(
            [WV_b[0].reshape(128, NC * 128), Ww_b, to_bf16(bwm)], axis=1
        )
        in_maps.append(
            {
                "A1": np.ascontiguousarray(A1),
                "hvT0": np.ascontiguousarray(hvT_b[0]),
                "A3": np.ascontiguousarray(A3),
                "WSr": WSr_b,
                "WVr": WVr_b,
                "hvTr": np.ascontiguousarray(hvT_b[1:].transpose(1, 0, 2, 3)),
                "bSV": bSV_b,
                "hs": to_bf16(hs_r),
            }
        )
    return in_maps


def run(inputs: dict, trace: bool = False):
    """Run on 8 NeuronCores; returns (output, BassKernelResults)."""
    from concourse import bass_utils

    order, bounds = _plan(inputs["lengths"])
    nc = _get_nc(bounds)
    in_maps = _make_in_maps(order, bounds, **inputs)
    res = bass_utils.run_bass_kernel_spmd(
        nc, in_maps, core_ids=list(range(NCORES)), trace=trace
    )
    full = np.zeros((B, T, D), dtype=np.float32)
    for core in range(NCORES):
        o = np.asarray(res.results[core]["out"], dtype=np.float32)
        for k in range(BPC):
            full[int(order[NCORES * k + core])] = o[k]
    return full, res


def kernel(**inputs) -> np.ndarray:
    out, _ = run(inputs, trace=False)
    return out
